# revision 4
# baseline (speedup 1.0000x reference)
"""Trainium2 Bass kernel for GrapherModule:
fc1+BN1 -> KNN(k=9, chunked-equivalent) -> MaxRelative conv+BN+GELU -> fc2+BN -> +residual.

Single fused SPMD kernel on 8 cores. Core d handles batch b=d//4 and query
slice qoff=(d%4)*2048: its x input is np.roll(x[b], -qoff) so queries are
always local nodes 0..2047 and the program is identical on every core.

On-device pipeline per core:
  A) x -> PE-transpose -> fc1 (fp32r matmuls) -> pre-h (feature-major,
     fp32r) + BN1 stats; tiny AllReduce for global stats; normalize in
     place; bias row brow[n] = -0.5*||h_n||^2 via ones-matmul.
  B) per 128-query tile: scores s = h_q . h_n + brow[n] computed fully in
     PE (fp32r matmul + rank-1 bias matmul accumulated in PSUM), ACT copies
     PSUM->SBUF, self-distance masked, DVE top-8 (max + max_index, uint16),
     idx DRAM round-trip into the gpsimd-wrapped layout, ap_gather of
     neighbor features from SBUF, DVE reduce_max over k=9 -> maxn.
  C) MaxRel conv + BN (AllReduce stats) + GELU, fc2 + BN (AllReduce),
     transpose + residual -> y.

BN statistics are exact over B*N tokens via AllReduce collectives.
"""
import sys, os
sys.path.insert(0, '/opt/trn_rl_repo')
os.environ.setdefault('JAX_PLATFORMS', 'cpu')

import numpy as np

B, N, C = 2, 8192, 128
K = 9
NQ = 2048          # queries per core
NT = NQ // 128     # 16 query tiles per core
EPS = 1e-5

_CACHE = {}


def _build():
    import concourse.bass as bass
    import concourse.mybir as mybir
    import concourse.tile as tile
    from concourse import bacc
    from concourse.masks import make_identity

    dt = mybir.dt
    AF = mybir.ActivationFunctionType
    ALU = mybir.AluOpType
    AX = mybir.AxisListType

    nc = bacc.Bacc("TRN2", target_bir_lowering=False, debug=False,
                   enable_asserts=False, num_devices=8)

    x_own = nc.dram_tensor("x_own", [N, C], dt.float32, kind="ExternalInput")
    fc1_w = nc.dram_tensor("fc1_w", [C, C], dt.float32, kind="ExternalInput")
    fc1_b = nc.dram_tensor("fc1_b", [C], dt.float32, kind="ExternalInput")
    bn1_g = nc.dram_tensor("bn1_g", [C], dt.float32, kind="ExternalInput")
    bn1_b = nc.dram_tensor("bn1_b", [C], dt.float32, kind="ExternalInput")
    conv_w = nc.dram_tensor("conv_w", [C, 2 * C], dt.float32, kind="ExternalInput")
    conv_b = nc.dram_tensor("conv_b", [C], dt.float32, kind="ExternalInput")
    bnc_g = nc.dram_tensor("bnc_g", [C], dt.float32, kind="ExternalInput")
    bnc_b = nc.dram_tensor("bnc_b", [C], dt.float32, kind="ExternalInput")
    fc2_w = nc.dram_tensor("fc2_w", [C, C], dt.float32, kind="ExternalInput")
    fc2_b = nc.dram_tensor("fc2_b", [C], dt.float32, kind="ExternalInput")
    bn2_g = nc.dram_tensor("bn2_g", [C], dt.float32, kind="ExternalInput")
    bn2_b = nc.dram_tensor("bn2_b", [C], dt.float32, kind="ExternalInput")
    y = nc.dram_tensor("y", [NQ, C], dt.float32, kind="ExternalOutput")
    idx_d = nc.dram_tensor("idx_d", [NT * 128 * K], dt.int16, kind="Internal")

    def col(t):  # [C] dram -> [C,1] view
        return t[:].rearrange("(c one) -> c one", one=1)

    with tile.TileContext(nc) as tc:
        wpool = tc.alloc_tile_pool(name="w", bufs=1)
        pers = tc.alloc_tile_pool(name="pers", bufs=1)
        dram = tc.alloc_tile_pool(name="dram", bufs=2, space="DRAM")

        ident = wpool.tile([128, 128], dt.float32)
        make_identity(nc, ident[:])

        # weights: DMA strided-transposed views, then round to fp32r
        def load_wT(view, tag):
            f = wpool.tile([C, C], dt.float32, tag="wtmp")
            nc.sync.dma_start(f[:], view)
            r = wpool.tile([C, C], dt.float32r, tag=tag)
            nc.vector.tensor_copy(r[:], f[:])
            return r

        fc1wT = load_wT(fc1_w[:].rearrange("m k -> k m"), "fc1wT")
        cw1T = load_wT(conv_w[:, 0:C].rearrange("m k -> k m"), "cw1T")
        cw2T = load_wT(conv_w[:, C:2 * C].rearrange("m k -> k m"), "cw2T")
        fc2wT = load_wT(fc2_w[:].rearrange("m k -> k m"), "fc2wT")

        def load_col(t, tag):
            c_ = wpool.tile([C, 1], dt.float32, tag=tag)
            nc.sync.dma_start(c_[:], col(t))
            return c_

        fc1b = load_col(fc1_b, "fc1b")
        bn1g = load_col(bn1_g, "bn1g")
        bn1bb = load_col(bn1_b, "bn1bb")
        convb = load_col(conv_b, "convb")
        bncg = load_col(bnc_g, "bncg")
        bncb = load_col(bnc_b, "bncb")
        fc2b = load_col(fc2_b, "fc2b")
        bn2g = load_col(bn2_g, "bn2g")
        bn2bb = load_col(bn2_b, "bn2bb")

        ones_f = wpool.tile([1, 128], dt.float32)
        nc.vector.memset(ones_f[:], 1.0)
        ones1r = wpool.tile([1, 128], dt.float32r)
        nc.vector.tensor_copy(ones1r[:], ones_f[:])
        mhalf_f = wpool.tile([128, 1], dt.float32)
        nc.vector.memset(mhalf_f[:], -0.5)
        mhalfr = wpool.tile([128, 1], dt.float32r)
        nc.vector.tensor_copy(mhalfr[:], mhalf_f[:])

        # ---------- AllReduce helper (stats over 8 cores) ----------
        def allreduce2(sump, ssqp):
            loc = pers.tile([128, 2], dt.float32, tag="arloc")
            nc.vector.reduce_sum(loc[:, 0:1], sump[:], axis=AX.X)
            nc.vector.reduce_sum(loc[:, 1:2], ssqp[:], axis=AX.X)
            bin_ = dram.tile([128, 2], dt.float32, tag="arin")
            bout = dram.tile([128, 2], dt.float32, tag="arout")
            nc.gpsimd.dma_start(bin_[:], loc[:])
            nc.gpsimd.collective_compute(
                "AllReduce", ALU.add, replica_groups=[list(range(8))],
                ins=[bin_.opt()], outs=[bout.opt()])
            tot = pers.tile([128, 2], dt.float32, tag="artot")
            nc.gpsimd.dma_start(tot[:], bout[:])
            return tot

        def bnparams(tot, gam, bet, count):
            st = pers.tile([128, 8], dt.float32, tag="bnst")
            mm, e2, vv, rr, sc, bi = (st[:, j:j + 1] for j in range(6))
            nc.vector.tensor_scalar_mul(mm, tot[:, 0:1], 1.0 / count)
            nc.vector.tensor_scalar_mul(e2, tot[:, 1:2], 1.0 / count)
            nc.vector.tensor_tensor(vv, mm, mm, op=ALU.mult)
            nc.vector.tensor_sub(vv, e2, vv)
            nc.vector.tensor_scalar(vv, vv, EPS, None, op0=ALU.add)
            nc.vector.reciprocal(rr, vv)
            nc.scalar.activation(rr, rr, AF.Sqrt)
            nc.vector.tensor_tensor(sc, rr, gam, op=ALU.mult)
            nc.vector.tensor_tensor(bi, mm, sc, op=ALU.mult)
            nc.vector.tensor_sub(bi, bet, bi)
            return sc, bi

        # ---------- Phase A: transpose x, fc1 (fp32r), BN1 stats ----------
        h = pers.tile([128, N], dt.float32r)     # pre-h, then h (in-place norm)
        sum_p = pers.tile([128, 16], dt.float32)
        ssq_p = pers.tile([128, 16], dt.float32)

        with tc.tile_pool(name="phA", bufs=3) as phA, \
             tc.tile_pool(name="phAj", bufs=2) as phAj, \
             tc.tile_pool(name="psT", bufs=4, space="PSUM") as psT, \
             tc.tile_pool(name="psF", bufs=2, space="PSUM") as psF:
            for g in range(16):            # groups of 4 token tiles (512 tokens)
                xTr = phA.tile([128, 512], dt.float32r, tag="xTr")
                for j in range(4):
                    r0 = g * 512 + j * 128
                    xt = phA.tile([128, 128], dt.float32, tag="xt")
                    nc.sync.dma_start(xt[:], x_own[r0:r0 + 128, :])
                    pxt = psT.tile([128, 128], dt.float32, tag="pT")
                    nc.tensor.transpose(pxt[:], xt[:], ident[:])
                    nc.vector.tensor_copy(xTr[:, j * 128:(j + 1) * 128], pxt[:])
                pre = psF.tile([128, 512], dt.float32, tag="pF")
                nc.tensor.matmul(pre[:], fc1wT[:], xTr[:], start=True, stop=True)
                sl = slice(g * 512, (g + 1) * 512)
                nc.scalar.activation(h[:, sl], pre[:], AF.Identity,
                                     bias=fc1b[:], accum_out=sum_p[:, g:g + 1])
                junk = phAj.tile([128, 512], dt.float32, tag="jq")
                nc.scalar.activation(junk[:], h[:, sl], AF.Square,
                                     accum_out=ssq_p[:, g:g + 1])

        sc1, bi1 = bnparams(allreduce2(sum_p, ssq_p), bn1g[:], bn1bb[:], 4 * B * N)
        # normalize pre-h in place (fp32r out)
        nc.scalar.activation(h[:], h[:], AF.Identity, bias=bi1, scale=sc1)

        # brow[n] = -0.5*||h_n||^2  (fp32r row), via Square + (-0.5 ones)-matmul
        brow = pers.tile([1, N], dt.float32r)
        with tc.tile_pool(name="nx", bufs=2) as nxp, \
             tc.tile_pool(name="psN", bufs=2, space="PSUM") as psN:
            for g in range(16):
                sl = slice(g * 512, (g + 1) * 512)
                h2 = nxp.tile([128, 512], dt.float32r, tag="h2")
                nc.scalar.activation(h2[:], h[:, sl], AF.Square)
                pn = psN.tile([1, 512], dt.float32, tag="pN")
                nc.tensor.matmul(pn[:], mhalfr[:], h2[:], start=True, stop=True)
                nc.scalar.activation(brow[0:1, sl], pn[:], AF.Identity)

        # ---------- Phase B: scores -> top-9 -> gather -> maxn ----------
        maxn = pers.tile([128, NQ], dt.float32)
        res_all = pers.tile([128, NQ], dt.float32)   # residual x, token-major
        with tc.tile_pool(name="sp", bufs=2) as sp, \
             tc.tile_pool(name="smal", bufs=4) as smal, \
             tc.tile_pool(name="gth", bufs=2) as gth, \
             tc.tile_pool(name="psB", bufs=2, space="PSUM") as psB:
            for i in range(NT):
                q0 = i * 128
                nc.sync.dma_start(res_all[:, q0:q0 + 128], x_own[q0:q0 + 128, :])
                s = sp.tile([128, N], dt.float32, tag="s")
                for g in range(4):
                    pg = psB.tile([128, 2048], dt.float32, tag="pg")
                    for c_ in range(4):
                        ch = slice((g * 4 + c_) * 512, (g * 4 + c_ + 1) * 512)
                        po = pg[:, c_ * 512:(c_ + 1) * 512]
                        nc.tensor.matmul(po, h[:, q0:q0 + 128], h[:, ch],
                                         start=True, stop=False)
                        nc.tensor.matmul(po, ones1r[:], brow[0:1, ch],
                                         start=False, stop=True)
                    nc.scalar.activation(s[:, g * 2048:(g + 1) * 2048], pg[:],
                                         AF.Identity)
                # mask self (diagonal of the query block)
                nc.gpsimd.affine_select(
                    s[:, q0:q0 + 128], s[:, q0:q0 + 128],
                    pattern=[[1, 128]], compare_op=ALU.not_equal,
                    fill=-1e30, base=0, channel_multiplier=-1)
                v8 = smal.tile([128, 8], dt.float32, tag="v8")
                nc.vector.max(v8[:], s[:])
                i9 = smal.tile([128, K], dt.uint16, tag="i9")
                nc.gpsimd.iota(i9[:, 0:1], pattern=[[0, 1]], base=q0,
                               channel_multiplier=1)
                nc.vector.max_index(i9[:, 1:9], v8[:], s[:])
                # idx round-trip: q-major flat -> wrapped [16, 72] x8
                fl = idx_d[i * 1152:(i + 1) * 1152]
                nc.sync.dma_start(
                    fl.rearrange("(q k) -> q k", k=K).bitcast(dt.uint16), i9[:])
                iw = gth.tile([128, 72], dt.int16, tag="iw")
                for c_ in range(8):
                    nc.sync.dma_start(iw[16 * c_:16 * c_ + 16, :],
                                      fl.rearrange("(f w) -> w f", w=16))
                gout = gth.tile([128, 128, K], dt.float32, tag="gout")
                nc.gpsimd.ap_gather(
                    gout[:].rearrange("p q k -> p (q k)"),
                    h[:].bitcast(dt.float32), iw[:],
                    channels=128, num_elems=N, d=1, num_idxs=128 * K)
                nc.vector.tensor_reduce(maxn[:, q0:q0 + 128], gout[:],
                                        axis=AX.X, op=ALU.max)

        # ---------- Phase C: MaxRel conv + BN + GELU, fc2 + BN, residual ----------
        convpre = pers.tile([128, NQ], dt.float32)
        csum_p = pers.tile([128, 4], dt.float32)
        cssq_p = pers.tile([128, 4], dt.float32)
        with tc.tile_pool(name="cj", bufs=2) as cj, \
             tc.tile_pool(name="psC", bufs=2, space="PSUM") as psC:
            for c_ in range(4):
                sl = slice(c_ * 512, (c_ + 1) * 512)
                r2 = cj.tile([128, 512], dt.float32r, tag="r2")
                nc.vector.tensor_sub(r2[:], maxn[:, sl], h[:, sl].bitcast(dt.float32))
                pc = psC.tile([128, 512], dt.float32, tag="pc")
                nc.tensor.matmul(pc[:], cw1T[:], h[:, sl], start=True, stop=False)
                nc.tensor.matmul(pc[:], cw2T[:], r2[:], start=False, stop=True)
                nc.scalar.activation(convpre[:, sl], pc[:], AF.Identity,
                                     bias=convb[:], accum_out=csum_p[:, c_:c_ + 1])
                jq = cj.tile([128, 512], dt.float32, tag="jq")
                nc.scalar.activation(jq[:], convpre[:, sl], AF.Square,
                                     accum_out=cssq_p[:, c_:c_ + 1])

        scc, bic = bnparams(allreduce2(csum_p, cssq_p), bncg[:], bncb[:], B * N)
        g_r = pers.tile([128, NQ], dt.float32r)
        nc.scalar.activation(g_r[:], convpre[:], AF.Gelu, bias=bic, scale=scc)

        f2pre = pers.tile([128, NQ], dt.float32)
        fsum_p = pers.tile([128, 4], dt.float32)
        fssq_p = pers.tile([128, 4], dt.float32)
        with tc.tile_pool(name="fj", bufs=2) as fj, \
             tc.tile_pool(name="psD", bufs=2, space="PSUM") as psD:
            for c_ in range(4):
                sl = slice(c_ * 512, (c_ + 1) * 512)
                pf = psD.tile([128, 512], dt.float32, tag="pf")
                nc.tensor.matmul(pf[:], fc2wT[:], g_r[:, sl], start=True, stop=True)
                nc.scalar.activation(f2pre[:, sl], pf[:], AF.Identity, bias=fc2b[:],
                                     accum_out=fsum_p[:, c_:c_ + 1])
                jf = fj.tile([128, 512], dt.float32, tag="jf")
                nc.scalar.activation(jf[:], f2pre[:, sl], AF.Square,
                                     accum_out=fssq_p[:, c_:c_ + 1])

        scf, bif = bnparams(allreduce2(fsum_p, fssq_p), bn2g[:], bn2bb[:], B * N)
        outfm = pers.tile([128, NQ], dt.float32)
        nc.scalar.activation(outfm[:], f2pre[:], AF.Identity, bias=bif, scale=scf)

        with tc.tile_pool(name="op", bufs=4) as op, \
             tc.tile_pool(name="psO", bufs=2, space="PSUM") as psO:
            for i in range(NT):
                q0 = i * 128
                po = psO.tile([128, 128], dt.float32, tag="po")
                nc.tensor.transpose(po[:], outfm[:, q0:q0 + 128], ident[:])
                ot = op.tile([128, 128], dt.float32, tag="ot")
                nc.vector.tensor_add(ot[:], po[:], res_all[:, q0:q0 + 128])
                nc.sync.dma_start(y[q0:q0 + 128, :], ot[:])

        for p in (dram, pers, wpool):
            p.release()

    nc.compile()
    return nc


def kernel(**inputs):
    from concourse import bass_utils

    if 'nc' not in _CACHE:
        _CACHE['nc'] = _build()
    nc = _CACHE['nc']

    f32 = lambda a: np.ascontiguousarray(np.asarray(a), dtype=np.float32)
    x = f32(inputs['x'])
    names = ['fc1_w', 'fc1_b', 'bn1_g', 'bn1_b', 'conv_w', 'conv_b',
             'bnc_g', 'bnc_b', 'fc2_w', 'fc2_b', 'bn2_g', 'bn2_b']
    w = {n: f32(inputs[n]) for n in names}

    in_maps = []
    for d in range(8):
        b, qoff = d // 4, (d % 4) * NQ
        m = dict(w)
        m['x_own'] = np.ascontiguousarray(np.roll(x[b], -qoff, axis=0))
        in_maps.append(m)

    r = bass_utils.run_bass_kernel_spmd(nc, in_maps, core_ids=list(range(8)))
    _CACHE['last_res'] = r

    out = np.empty((B, N, C), np.float32)
    for d in range(8):
        b, qoff = d // 4, (d % 4) * NQ
        out[b, qoff:qoff + NQ] = r.results[d]['y']
    return out
